# revision 33
# baseline (speedup 1.0000x reference)
"""RGCN (2-layer, mean-aggregation) Bass kernel for one TRN2 chip (8 NeuronCores).

Strategy (dst-sharded, matmul-based aggregation — no DRAM scatter):
  - Nodes block-partitioned across 8 cores (12500/core, padded to 12544).
    Edges live on their dst-owner core; x is replicated (bf16, padded layout
    [C*NLP, D]) in every core's HBM.
  - Edges are sorted by (dst tile, src-quarter, rel-group) and padded per
    group to a multiple of 128. Per (tile, src-quarter) run: dma_gather the
    messages x[src] ([128e, D] natural layout), scale by inv_deg (DVE), build
    a one-hot matrix M[e, (rel%GRP)*128 + dst%128] (DVE is_equal against a
    resident iota row, or DMA a host-precomputed fp8 M when K_MDRAM=1), then
    aggregate on the tensor engine:
        A_grpT[d, code] += msg^T @ M   (PSUM, f32 accumulation)
    A_grpT is directly the lhsT for the transform matmuls:
        out[dst, :] = relu(sum_r A_rT.T @ W_r + x_locT.T @ Wroot + b)
  - PSUM: each accumulator is padded to a full 2KB bank (PE start=True zeroes
    the whole 2KB region, so concurrently-open groups must not share a bank);
    accumulators are double-buffered and the transform output aliases into a
    closed accumulator bank.
  - Gathers use int16 indices over 4 equal row-quarters of the replicated
    table (25088 rows < 32768), one SWDGE queue per quarter; deep tile-pool
    buffering (bufs=8) keeps a full tile of 4 gather runs in flight.
  - Between layers one small AllGather (bf16, 3.2MB/rank) replicates the new
    features; layer 1's replication is free (host pre-stages x_rep).
"""

import os
import numpy as np
import ml_dtypes

import concourse.tile as tile
from concourse import bass, bacc, mybir
from concourse.bass_utils import run_bass_kernel_spmd

BF16 = mybir.dt.bfloat16
F32 = mybir.dt.float32
I16 = mybir.dt.int16
bf16 = ml_dtypes.bfloat16

K_QUEUES = int(os.environ.get("K_QUEUES", "4"))
SINGLE_PACKET = os.environ.get("K_SP", "0") == "1"
MAXC = int(os.environ.get("K_MAXC", "1024" if SINGLE_PACKET else "1920"))
# M source: 0 = build one-hot on DVE (is_equal), 1 = DMA precomputed fp8,
# 2 = hybrid (even src-quarters DMA, odd src-quarters DVE)
K_MDRAM = int(os.environ.get("K_MDRAM", "2"))
K_GRP = int(os.environ.get("K_GRP", "2"))   # relations per accumulator group
K_B = int(os.environ.get("K_B", "1"))       # dst tiles per batch
FP8 = mybir.dt.float8e4
fp8 = ml_dtypes.float8_e4m3fn

# ----------------------------------------------------------------------------
# Problem constants
# ----------------------------------------------------------------------------
FULL = dict(N=100000, E=1000000, D=128, R=8, C=8)


def derive(cfg):
    N, D, R, C = cfg["N"], cfg["D"], cfg["R"], cfg["C"]
    NL = N // C                      # owned nodes per core
    NT = (NL + 127) // 128           # dst tiles per core
    NLP = NT * 128                   # padded rows per block
    NTOT = C * NLP                   # replicated-table rows
    B = K_B                          # dst tiles per batch
    NBAT = NT // B                   # batches per core
    PAIRS = R // K_GRP               # accumulator groups per (tile)
    NSB = 4                          # src table quarters
    SBR = NTOT // NSB                # rows per quarter (must be < 32768)
    MW = K_GRP * 128                 # M width: (rel%GRP)*128 + dst%128
    # PSUM: PAIRS*B accumulators, each padded to a full 2KB bank, x2 bufs
    assert PAIRS * B * 2 <= 8 and MW <= 512
    return NL, NT, NLP, NTOT, B, NBAT, PAIRS, NSB, SBR, MW


# ----------------------------------------------------------------------------
# Host-side preprocessing
# ----------------------------------------------------------------------------
def host_prep(x, edge_index, edge_type, cfg):
    N, E, D, R, C = cfg["N"], cfg["E"], cfg["D"], cfg["R"], cfg["C"]
    NL, NT, NLP, NTOT, B, NBAT, PAIRS, NSB, SBR, MW = derive(cfg)
    assert SBR < 32768 and NT % B == 0

    src = np.asarray(edge_index[0], dtype=np.int64)
    dst = np.asarray(edge_index[1], dtype=np.int64)
    et = np.asarray(edge_type, dtype=np.int64)

    # mean-normalization per (relation, dst), computed on host (graph-only)
    deg = np.zeros((R, N), np.float32)
    np.add.at(deg, (et, dst), 1.0)
    inv = np.where(deg > 0, 1.0 / np.maximum(deg, 1.0), 0.0).astype(np.float32)
    scale_e = inv[et, dst]

    core = dst // NL
    dl = dst % NL
    tl = dl // 128
    bat = tl // B
    t2 = tl % B
    pair = et // K_GRP
    code = (et % K_GRP) * 128 + (dl % 128)
    srcp = (src // NL) * NLP + (src % NL)       # padded replicated-table row
    sb = srcp // SBR
    sidx = (srcp % SBR).astype(np.int64)

    NG = NBAT * NSB * PAIRS * B
    g = ((bat * NSB + sb) * PAIRS + pair) * B + t2

    counts = np.zeros((C, NG), np.int64)
    np.add.at(counts, (core, g), 1)
    gsz = np.maximum(((counts.max(axis=0) + 127) // 128) * 128, 128)  # [NG]
    offs = np.zeros(NG + 1, np.int64)
    np.cumsum(gsz, out=offs[1:])
    PAD = int(offs[-1])

    # place edges: stable sort by (core, g), rank within each (core, g) run
    key = core * NG + g
    order = np.argsort(key, kind="stable")
    key_o = key[order]
    new_run = np.ones(E, bool)
    new_run[1:] = key_o[1:] != key_o[:-1]
    run_starts = np.flatnonzero(new_run)
    run_id = np.cumsum(new_run) - 1
    rank = np.arange(E) - run_starts[run_id]
    pos = offs[g[order]] + rank

    gidx_a = np.zeros((C, PAD), np.int16)
    code_i = np.full((C, PAD), 10000, np.int32)   # pad sentinel, out of range
    scal_a = np.zeros((C, PAD), bf16)
    co = core[order]
    gidx_a[co, pos] = sidx[order].astype(np.int16)
    code_i[co, pos] = code[order].astype(np.int32)
    scal_a[co, pos] = scale_e[order].astype(bf16)
    # bf16 codes only feed the is_equal path; bf16 integers are exact to
    # 256 so that path requires MW <= 256
    assert K_MDRAM == 1 or MW <= 256
    code_a = np.minimum(code_i, 300).astype(bf16)

    # wrapped layouts: idx i at [i%16, i//16] (x8 down); code/scale at
    # [i%128, i//128]
    gidx_w = np.tile(
        gidx_a.reshape(C, PAD // 16, 16).transpose(0, 2, 1), (1, 8, 1)
    )
    dstv_w = np.ascontiguousarray(
        code_a.reshape(C, PAD // 128, 128).transpose(0, 2, 1)
    )
    scal_w = np.ascontiguousarray(
        scal_a.reshape(C, PAD // 128, 128).transpose(0, 2, 1)
    )

    # precomputed one-hot M (fp8), wrapped [C, 128, (PAD//128)*MW]
    m_w = None
    if K_MDRAM:
        m_full = (
            code_i.reshape(C, PAD // 128, 128)[..., None]
            == np.arange(MW, dtype=np.int32)
        )
        m_w = np.ascontiguousarray(
            m_full.transpose(0, 2, 1, 3).reshape(C, 128, (PAD // 128) * MW)
        ).astype(fp8)

    # replicated, block-padded x (bf16): [NTOT, D]
    x = np.asarray(x, np.float32)
    x_rep = np.zeros((NTOT, D), bf16)
    for c in range(C):
        x_rep[c * NLP : c * NLP + NL] = x[c * NL : (c + 1) * NL].astype(bf16)

    # run table: per (bat, sb): (offset, size); groups within are (pair, t2)
    runs = []
    for b_ in range(NBAT):
        row = []
        for s_ in range(NSB):
            g0 = ((b_ * NSB + s_) * PAIRS) * B
            o = int(offs[g0])
            n = int(offs[g0 + PAIRS * B] - offs[g0])
            row.append((o, n))
        runs.append(row)

    return dict(
        gsz=tuple(int(v) for v in gsz),
        runs=tuple(tuple(r) for r in runs),
        PAD=PAD,
        gidx=np.ascontiguousarray(gidx_w),
        dstv=dstv_w,
        scal=scal_w,
        m_w=m_w,
        code_i=code_i,
        x_rep=x_rep,
    )


# ----------------------------------------------------------------------------
# Device program
# ----------------------------------------------------------------------------
def build_program(cfg, gsz, runs, PAD):
    N, E, D, R, C = cfg["N"], cfg["E"], cfg["D"], cfg["R"], cfg["C"]
    NL, NT, NLP, NTOT, B, NBAT, PAIRS, NSB, SBR, MW = derive(cfg)

    nc = bacc.Bacc(
        "TRN2", target_bir_lowering=False, debug=False,
        enable_asserts=False, num_devices=C, num_swdge_queues=K_QUEUES,
    )

    x_rep = nc.dram_tensor("x_rep", [NTOT, D], BF16, kind="ExternalInput")
    x_loc = nc.dram_tensor("x_loc", [NLP, D], BF16, kind="ExternalInput")
    w_all = nc.dram_tensor("w_all", [2, R + 1, D, D], BF16, kind="ExternalInput")
    b_all = nc.dram_tensor("b_all", [2, 1, D], BF16, kind="ExternalInput")
    gidx_d = nc.dram_tensor("gidx", [128, PAD // 16], I16, kind="ExternalInput")
    dstv_d = nc.dram_tensor("dstv", [128, PAD // 128], BF16, kind="ExternalInput")
    scal_d = nc.dram_tensor("scal", [128, PAD // 128], BF16, kind="ExternalInput")
    ciota_d = nc.dram_tensor("ciota", [128, MW], BF16, kind="ExternalInput")
    if K_MDRAM:
        m_d = nc.dram_tensor(
            "m_w", [128, (PAD // 128) * MW], FP8, kind="ExternalInput"
        )
    out_d = nc.dram_tensor("out", [NL, D], F32, kind="ExternalOutput")
    h1b = nc.dram_tensor("h1b", [NLP, D], BF16, kind="Internal")
    h1rep = nc.dram_tensor(
        "h1rep", [NTOT, D], BF16, kind="Internal", addr_space="Shared"
    )

    with tile.TileContext(nc) as tc:
        with (
            tc.tile_pool(name="resident", bufs=1) as res_pool,
            tc.tile_pool(name="msg", bufs=8) as msg_pool,
            tc.tile_pool(name="mm", bufs=8) as m_pool,
            tc.tile_pool(name="asb", bufs=3) as a_pool,
            tc.tile_pool(name="loct", bufs=4) as loct_pool,
            tc.tile_pool(name="wpool", bufs=1) as wpool,
            tc.tile_pool(name="hout", bufs=8) as hpool,
            tc.tile_pool(name="psA", bufs=1, space="PSUM") as psA_pool,
        ):
            gidx_sb = res_pool.tile([128, PAD // 16], I16)
            dstv_sb = res_pool.tile([128, PAD // 128], BF16)
            scal_sb = res_pool.tile([128, PAD // 128], BF16)
            ciota_sb = res_pool.tile([128, MW], BF16)
            nc.sync.dma_start(out=gidx_sb[:], in_=gidx_d.ap()[:, :])
            nc.sync.dma_start(out=dstv_sb[:], in_=dstv_d.ap()[:, :])
            nc.sync.dma_start(out=scal_sb[:], in_=scal_d.ap()[:, :])
            nc.sync.dma_start(out=ciota_sb[:], in_=ciota_d.ap()[:, :])
            ones_sb = res_pool.tile([1, D], BF16)
            nc.vector.memset(ones_sb[:], 1.0)

            for lay in range(2):
                src_tab = x_rep if lay == 0 else h1rep
                loc_tab = x_loc if lay == 0 else h1b

                ls = nc.enter_named_scope(f"lay_{lay}", False)
                w_sb = wpool.tile([128, (R + 1) * D], BF16, tag="w", bufs=2)
                nc.sync.dma_start(
                    out=w_sb[:].rearrange("d (r e) -> d r e", r=R + 1),
                    in_=w_all.ap()[lay].rearrange("r d e -> d r e"),
                )
                b_sb = wpool.tile([1, D], BF16, tag="b", bufs=2)
                nc.sync.dma_start(out=b_sb[:], in_=b_all.ap()[lay])

                NACC = PAIRS * B
                for bat in range(NBAT):
                    row0 = bat * B * 128
                    if B > 1 or bat % 2 == 0:
                        nrows = min(max(B, 2) * 128, NLP - row0)
                        loct = loct_pool.tile([128, max(B, 2) * 128], BF16,
                                              tag="lt")
                        nc.sync.dma_start_transpose(
                            out=loct[:, :nrows],
                            in_=loc_tab.ap()[row0 : row0 + nrows, :],
                        )
                    # accumulators, each padded to a full PSUM bank so every
                    # concurrently-open accumulation group owns its own 2KB
                    # zero region (PE start=True zeroes the whole region)
                    psA = [
                        psA_pool.tile([128, MW], F32, tag=f"pa{a}",
                                      name=f"psA{a}", bufs=2,
                                      padded_shape=[128, 512])
                        for a in range(NACC)
                    ]
                    for sb in range(NSB):
                        o, n = runs[bat][sb]
                        nch = n // 128
                        msgt = msg_pool.tile([128, nch, D], BF16, tag="msg")
                        for co in range(0, n, MAXC):
                            cn = min(MAXC, n - co)
                            nc.gpsimd.dma_gather(
                                out_ap=msgt[:, co // 128 : (co + cn) // 128, :],
                                in_ap=src_tab.ap()[sb * SBR : (sb + 1) * SBR, :],
                                idxs_ap=gidx_sb[
                                    :, (o + co) // 16 : (o + co + cn) // 16
                                ],
                                num_idxs=cn,
                                num_idxs_reg=cn,
                                elem_size=D,
                                single_packet=SINGLE_PACKET,
                                queue_num=sb % K_QUEUES,
                            )
                        use_md = (
                            K_MDRAM == 1
                            or (K_MDRAM == 2 and sb % 2 == 0)
                            or (K_MDRAM == 3 and sb != 1)
                        )
                        if use_md:
                            mt = m_pool.tile([128, nch * MW], FP8, tag="mf8")
                            nc.scalar.dma_start(
                                out=mt[:],
                                in_=m_d.ap()[
                                    :, (o // 128) * MW : ((o + n) // 128) * MW
                                ],
                            )
                        else:
                            # is_equal first: it does not depend on the
                            # gather, so it must not sit behind the
                            # gather-gated scale in the DVE queue
                            mt = m_pool.tile([128, nch * MW], BF16, tag="m16")
                            nc.vector.tensor_tensor(
                                out=mt[:].rearrange(
                                    "p (a b) -> p a b", b=MW
                                ),
                                in0=dstv_sb[:, o // 128 : (o + n) // 128, None]
                                .to_broadcast([128, nch, MW]),
                                in1=ciota_sb[:, None, :]
                                .to_broadcast([128, nch, MW]),
                                op=mybir.AluOpType.is_equal,
                            )
                        nc.vector.tensor_tensor(
                            out=msgt[:],
                            in0=msgt[:],
                            in1=scal_sb[:, o // 128 : (o + n) // 128, None]
                            .to_broadcast([128, nch, D]),
                            op=mybir.AluOpType.mult,
                        )
                        ci = 0
                        for p in range(PAIRS):
                            for t2 in range(B):
                                gi = ((bat * NSB + sb) * PAIRS + p) * B + t2
                                gch = gsz[gi] // 128
                                for k in range(gch):
                                    nc.tensor.matmul(
                                        out=psA[p * B + t2][:],
                                        lhsT=msgt[:, ci, :],
                                        rhs=mt[:, ci * MW : (ci + 1) * MW],
                                        start=(sb == 0 and k == 0),
                                        stop=(sb == NSB - 1 and k == gch - 1),
                                    )
                                    ci += 1
                        assert ci == nch

                    a_sb = [
                        a_pool.tile([128, MW], BF16, tag=f"a{a}",
                                    name=f"a_sb{a}", bufs=3)
                        for a in range(NACC)
                    ]
                    for a in range(NACC):
                        nc.scalar.activation(
                            out=a_sb[a][:], in_=psA[a][:],
                            func=mybir.ActivationFunctionType.Copy,
                        )

                    for t2 in range(B):
                        # transform reuses a late accumulator's bank (its
                        # group is closed and its data copied to SBUF by now)
                        ps = psA[(PAIRS - 1) * B + t2][:, 0:D]
                        for r in range(R):
                            a0 = (r % K_GRP) * 128
                            nc.tensor.matmul(
                                out=ps,
                                lhsT=a_sb[(r // K_GRP) * B + t2][:, a0 : a0 + 128],
                                rhs=w_sb[:, r * D : (r + 1) * D],
                                start=(r == 0),
                                stop=False,
                            )
                        lc = (bat % 2) * 128 if B == 1 else t2 * 128
                        nc.tensor.matmul(
                            out=ps,
                            lhsT=loct[:, lc : lc + 128],
                            rhs=w_sb[:, R * D : (R + 1) * D],
                            start=False,
                            stop=False,
                        )
                        nc.tensor.matmul(
                            out=ps,
                            lhsT=ones_sb[:1, :],
                            rhs=b_sb[:1, :],
                            start=False,
                            stop=True,
                        )
                        row = row0 + t2 * 128
                        if lay == 0:
                            hs = hpool.tile([128, D], BF16, tag="h0")
                            nc.scalar.activation(
                                out=hs[:], in_=ps,
                                func=mybir.ActivationFunctionType.Relu,
                            )
                            nc.sync.dma_start(
                                out=h1b.ap()[row : row + 128, :], in_=hs[:]
                            )
                        else:
                            nrow = min(128, NL - row)
                            if nrow <= 0:
                                continue
                            hs = hpool.tile([128, D], F32, tag="h1")
                            nc.scalar.activation(
                                out=hs[:], in_=ps,
                                func=mybir.ActivationFunctionType.Relu,
                            )
                            nc.sync.dma_start(
                                out=out_d.ap()[row : row + nrow, :],
                                in_=hs[:nrow, :],
                            )

                nc.leave_named_scope(f"lay_{lay}", ls[0], False)
                if lay == 0:
                    nc.gpsimd.collective_compute(
                        "AllGather",
                        mybir.AluOpType.bypass,
                        replica_groups=[list(range(C))],
                        ins=[h1b.ap()],
                        outs=[h1rep.ap()],
                    )

    nc.compile()
    return nc


# ----------------------------------------------------------------------------
# In-map assembly
# ----------------------------------------------------------------------------
def make_in_maps(prep, W1, root1, b1, W2, root2, b2, cfg):
    C, D, R = cfg["C"], cfg["D"], cfg["R"]
    NL, NT, NLP, NTOT, B, NBAT, PAIRS, NSB, SBR, MW = derive(cfg)
    w_all = np.zeros((2, R + 1, D, D), bf16)
    w_all[0, :R] = np.asarray(W1, np.float32).astype(bf16)
    w_all[0, R] = np.asarray(root1, np.float32).astype(bf16)
    w_all[1, :R] = np.asarray(W2, np.float32).astype(bf16)
    w_all[1, R] = np.asarray(root2, np.float32).astype(bf16)
    b_stack = np.stack([np.asarray(b1, np.float32), np.asarray(b2, np.float32)])
    b_all = b_stack.reshape(2, 1, D).astype(bf16)
    ciota = np.tile(np.arange(MW, dtype=np.float32).astype(bf16), (128, 1))

    in_maps = []
    for c in range(C):
        x_loc = np.ascontiguousarray(prep["x_rep"][c * NLP : (c + 1) * NLP])
        im = {
            "x_rep": prep["x_rep"],
            "x_loc": x_loc,
            "w_all": w_all,
            "b_all": b_all,
            "gidx": prep["gidx"][c],
            "dstv": prep["dstv"][c],
            "scal": prep["scal"][c],
            "ciota": ciota,
        }
        if K_MDRAM:
            im["m_w"] = prep["m_w"][c]
        in_maps.append(im)
    return in_maps


def enable_ntff_hook():
    """Register the axon NTFF profiling hook if the image's antenv lacks it."""
    import sys, types
    try:
        import antenv.axon_hooks  # noqa: F401
        return True
    except ImportError:
        pass
    try:
        from trn_agent_boot.trn_boot import _ntff_profile_via_ctypes
        hook = _ntff_profile_via_ctypes("/opt/axon/libaxon_pjrt.so")
        mod = types.ModuleType("antenv.axon_hooks")
        mod._hook = hook
        mod.set_axon_ntff_profile_hook = lambda h: setattr(mod, "_hook", h)
        mod.get_axon_ntff_profile_hook = lambda: mod._hook
        sys.modules["antenv.axon_hooks"] = mod
        import antenv
        antenv.axon_hooks = mod
        return hook is not None
    except Exception:
        return False


_program_cache = {}


def run(x, edge_index, edge_type, W1, root1, b1, W2, root2, b2,
        cfg=FULL, trace=False):
    prep = host_prep(x, edge_index, edge_type, cfg)
    key = (tuple(sorted(cfg.items())), prep["gsz"], prep["runs"], prep["PAD"],
           K_QUEUES, SINGLE_PACKET, MAXC, K_MDRAM, K_GRP, K_B)
    if key not in _program_cache:
        _program_cache[key] = build_program(
            cfg, prep["gsz"], prep["runs"], prep["PAD"]
        )
    nc = _program_cache[key]
    in_maps = make_in_maps(prep, W1, root1, b1, W2, root2, b2, cfg)
    if trace:
        trace = enable_ntff_hook()
    res = run_bass_kernel_spmd(
        nc, in_maps, core_ids=list(range(cfg["C"])), trace=trace
    )
    blocks = [res.results[c]["out"] for c in range(cfg["C"])]
    full = np.concatenate(blocks, axis=0).astype(np.float32)
    return full, res


def kernel(**inputs):
    out, _ = run(
        inputs["x"], inputs["edge_index"], inputs["edge_type"],
        inputs["W1"], inputs["root1"], inputs["b1"],
        inputs["W2"], inputs["root2"], inputs["b2"],
    )
    return out


# revision 35
# speedup vs baseline: 1.1960x; 1.1960x over previous
"""RGCN (2-layer, mean-aggregation) Bass kernel for one TRN2 chip (8 NeuronCores).

Strategy (dst-sharded, matmul-based aggregation — no DRAM scatter):
  - Nodes block-partitioned across 8 cores (12500/core, padded to 12544).
    Edges live on their dst-owner core; x is replicated (bf16, padded layout
    [C*NLP, D]) in every core's HBM.
  - Edges are sorted by (dst tile, src-quarter, rel-group) and padded per
    group to a multiple of 128. Per (tile, src-quarter) run: dma_gather the
    messages x[src] ([128e, D] natural layout), scale by inv_deg (DVE), build
    a one-hot matrix M[e, (rel%GRP)*128 + dst%128] (DVE is_equal against a
    resident iota row, or DMA a host-precomputed fp8 M when K_MDRAM=1), then
    aggregate on the tensor engine:
        A_grpT[d, code] += msg^T @ M   (PSUM, f32 accumulation)
    A_grpT is directly the lhsT for the transform matmuls:
        out[dst, :] = relu(sum_r A_rT.T @ W_r + x_locT.T @ Wroot + b)
  - PSUM: each accumulator is padded to a full 2KB bank (PE start=True zeroes
    the whole 2KB region, so concurrently-open groups must not share a bank);
    accumulators are double-buffered and the transform output aliases into a
    closed accumulator bank.
  - Gathers use int16 indices over 4 equal row-quarters of the replicated
    table (25088 rows < 32768), one SWDGE queue per quarter; deep tile-pool
    buffering (bufs=8) keeps a full tile of 4 gather runs in flight.
  - Between layers one small AllGather (bf16, 3.2MB/rank) replicates the new
    features; layer 1's replication is free (host pre-stages x_rep).
"""

import os
import numpy as np
import ml_dtypes

import concourse.tile as tile
from concourse import bass, bacc, mybir
from concourse.bass_utils import run_bass_kernel_spmd

BF16 = mybir.dt.bfloat16
F32 = mybir.dt.float32
I16 = mybir.dt.int16
bf16 = ml_dtypes.bfloat16

K_QUEUES = int(os.environ.get("K_QUEUES", "4"))
SINGLE_PACKET = os.environ.get("K_SP", "0") == "1"
MAXC = int(os.environ.get("K_MAXC", "1024" if SINGLE_PACKET else "1920"))
# M source: 0 = build one-hot on DVE (is_equal), 1 = DMA precomputed fp8,
# 2 = hybrid (even src-quarters DMA, odd src-quarters DVE)
K_MDRAM = int(os.environ.get("K_MDRAM", "2"))
K_GRP = int(os.environ.get("K_GRP", "2"))   # relations per accumulator group
K_B = int(os.environ.get("K_B", "1"))       # dst tiles per batch
FP8 = mybir.dt.float8e4
fp8 = ml_dtypes.float8_e4m3fn

# ----------------------------------------------------------------------------
# Problem constants
# ----------------------------------------------------------------------------
FULL = dict(N=100000, E=1000000, D=128, R=8, C=8)


def derive(cfg):
    N, D, R, C = cfg["N"], cfg["D"], cfg["R"], cfg["C"]
    NL = N // C                      # owned nodes per core
    NT = (NL + 127) // 128           # dst tiles per core
    NLP = NT * 128                   # padded rows per block
    NTOT = C * NLP                   # replicated-table rows
    B = K_B                          # dst tiles per batch
    NBAT = NT // B                   # batches per core
    PAIRS = R // K_GRP               # accumulator groups per (tile)
    NSB = 4                          # src table quarters
    SBR = NTOT // NSB                # rows per quarter (must be < 32768)
    MW = K_GRP * 128                 # M width: (rel%GRP)*128 + dst%128
    # PSUM: PAIRS*B accumulators, each padded to a full 2KB bank, x2 bufs
    assert PAIRS * B * 2 <= 8 and MW <= 512
    return NL, NT, NLP, NTOT, B, NBAT, PAIRS, NSB, SBR, MW


# ----------------------------------------------------------------------------
# Host-side preprocessing
# ----------------------------------------------------------------------------
def host_prep(x, edge_index, edge_type, cfg):
    N, E, D, R, C = cfg["N"], cfg["E"], cfg["D"], cfg["R"], cfg["C"]
    NL, NT, NLP, NTOT, B, NBAT, PAIRS, NSB, SBR, MW = derive(cfg)
    assert SBR < 32768 and NT % B == 0

    src = np.asarray(edge_index[0], dtype=np.int64)
    dst = np.asarray(edge_index[1], dtype=np.int64)
    et = np.asarray(edge_type, dtype=np.int64)

    # mean-normalization per (relation, dst), computed on host (graph-only)
    deg = np.zeros((R, N), np.float32)
    np.add.at(deg, (et, dst), 1.0)
    inv = np.where(deg > 0, 1.0 / np.maximum(deg, 1.0), 0.0).astype(np.float32)
    scale_e = inv[et, dst]

    core = dst // NL
    dl = dst % NL
    tl = dl // 128
    bat = tl // B
    t2 = tl % B
    pair = et // K_GRP
    code = (et % K_GRP) * 128 + (dl % 128)
    srcp = (src // NL) * NLP + (src % NL)       # padded replicated-table row
    sb = srcp // SBR
    sidx = (srcp % SBR).astype(np.int64)

    NG = NBAT * NSB * PAIRS * B
    g = ((bat * NSB + sb) * PAIRS + pair) * B + t2

    counts = np.zeros((C, NG), np.int64)
    np.add.at(counts, (core, g), 1)
    gsz = np.maximum(((counts.max(axis=0) + 127) // 128) * 128, 128)  # [NG]
    offs = np.zeros(NG + 1, np.int64)
    np.cumsum(gsz, out=offs[1:])
    PAD = int(offs[-1])

    # place edges: stable sort by (core, g), rank within each (core, g) run
    key = core * NG + g
    order = np.argsort(key, kind="stable")
    key_o = key[order]
    new_run = np.ones(E, bool)
    new_run[1:] = key_o[1:] != key_o[:-1]
    run_starts = np.flatnonzero(new_run)
    run_id = np.cumsum(new_run) - 1
    rank = np.arange(E) - run_starts[run_id]
    pos = offs[g[order]] + rank

    gidx_a = np.zeros((C, PAD), np.int16)
    code_i = np.full((C, PAD), 10000, np.int32)   # pad sentinel, out of range
    scal_a = np.zeros((C, PAD), bf16)
    co = core[order]
    gidx_a[co, pos] = sidx[order].astype(np.int16)
    code_i[co, pos] = code[order].astype(np.int32)
    scal_a[co, pos] = scale_e[order].astype(bf16)
    # bf16 codes only feed the is_equal path; bf16 integers are exact to
    # 256 so that path requires MW <= 256
    assert K_MDRAM == 1 or MW <= 256
    code_a = np.minimum(code_i, 300).astype(bf16)

    # wrapped layouts: idx i at [i%16, i//16] (x8 down); code/scale at
    # [i%128, i//128]
    gidx_w = np.tile(
        gidx_a.reshape(C, PAD // 16, 16).transpose(0, 2, 1), (1, 8, 1)
    )
    dstv_w = np.ascontiguousarray(
        code_a.reshape(C, PAD // 128, 128).transpose(0, 2, 1)
    )
    scal_w = np.ascontiguousarray(
        scal_a.reshape(C, PAD // 128, 128).transpose(0, 2, 1)
    )

    # precomputed one-hot M (fp8), wrapped [C, 128, (PAD//128)*MW]
    m_w = None
    if K_MDRAM:
        m_full = (
            code_i.reshape(C, PAD // 128, 128)[..., None]
            == np.arange(MW, dtype=np.int32)
        )
        m_w = np.ascontiguousarray(
            m_full.transpose(0, 2, 1, 3).reshape(C, 128, (PAD // 128) * MW)
        ).astype(fp8)

    # replicated, block-padded x (bf16): [NTOT, D]
    x = np.asarray(x, np.float32)
    x_rep = np.zeros((NTOT, D), bf16)
    for c in range(C):
        x_rep[c * NLP : c * NLP + NL] = x[c * NL : (c + 1) * NL].astype(bf16)

    # run table: per (bat, sb): (offset, size); groups within are (pair, t2)
    runs = []
    for b_ in range(NBAT):
        row = []
        for s_ in range(NSB):
            g0 = ((b_ * NSB + s_) * PAIRS) * B
            o = int(offs[g0])
            n = int(offs[g0 + PAIRS * B] - offs[g0])
            row.append((o, n))
        runs.append(row)

    return dict(
        gsz=tuple(int(v) for v in gsz),
        runs=tuple(tuple(r) for r in runs),
        PAD=PAD,
        gidx=np.ascontiguousarray(gidx_w),
        dstv=dstv_w,
        scal=scal_w,
        m_w=m_w,
        code_i=code_i,
        x_rep=x_rep,
    )


# ----------------------------------------------------------------------------
# Device program
# ----------------------------------------------------------------------------
def build_program(cfg, gsz, runs, PAD):
    N, E, D, R, C = cfg["N"], cfg["E"], cfg["D"], cfg["R"], cfg["C"]
    NL, NT, NLP, NTOT, B, NBAT, PAIRS, NSB, SBR, MW = derive(cfg)

    nc = bacc.Bacc(
        "TRN2", target_bir_lowering=False, debug=False,
        enable_asserts=False, num_devices=C, num_swdge_queues=K_QUEUES,
    )

    x_rep = nc.dram_tensor("x_rep", [NTOT, D], BF16, kind="ExternalInput")
    x_loc = nc.dram_tensor("x_loc", [NLP, D], BF16, kind="ExternalInput")
    w_all = nc.dram_tensor("w_all", [2, R + 1, D, D], BF16, kind="ExternalInput")
    b_all = nc.dram_tensor("b_all", [2, 1, D], BF16, kind="ExternalInput")
    gidx_d = nc.dram_tensor("gidx", [128, PAD // 16], I16, kind="ExternalInput")
    dstv_d = nc.dram_tensor("dstv", [128, PAD // 128], BF16, kind="ExternalInput")
    scal_d = nc.dram_tensor("scal", [128, PAD // 128], BF16, kind="ExternalInput")
    ciota_d = nc.dram_tensor("ciota", [128, MW], BF16, kind="ExternalInput")
    if K_MDRAM:
        m_d = nc.dram_tensor(
            "m_w", [128, (PAD // 128) * MW], FP8, kind="ExternalInput"
        )
    out_d = nc.dram_tensor("out", [NL, D], F32, kind="ExternalOutput")
    h1b = nc.dram_tensor("h1b", [NLP, D], BF16, kind="Internal")
    h1rep = nc.dram_tensor(
        "h1rep", [NTOT, D], BF16, kind="Internal", addr_space="Shared"
    )

    with tile.TileContext(nc) as tc:
        with (
            tc.tile_pool(name="resident", bufs=1) as res_pool,
            tc.tile_pool(name="msg", bufs=8) as msg_pool,
            tc.tile_pool(name="mm", bufs=8) as m_pool,
            tc.tile_pool(name="asb", bufs=2) as a_pool,
            tc.tile_pool(name="loct", bufs=2) as loct_pool,
            tc.tile_pool(name="wpool", bufs=1) as wpool,
            tc.tile_pool(name="hout", bufs=4) as hpool,
            tc.tile_pool(name="psA", bufs=1, space="PSUM") as psA_pool,
        ):
            gidx_sb = res_pool.tile([128, PAD // 16], I16)
            dstv_sb = res_pool.tile([128, PAD // 128], BF16)
            scal_sb = res_pool.tile([128, PAD // 128], BF16)
            ciota_sb = res_pool.tile([128, MW], BF16)
            nc.sync.dma_start(out=gidx_sb[:], in_=gidx_d.ap()[:, :])
            nc.sync.dma_start(out=dstv_sb[:], in_=dstv_d.ap()[:, :])
            nc.sync.dma_start(out=scal_sb[:], in_=scal_d.ap()[:, :])
            nc.sync.dma_start(out=ciota_sb[:], in_=ciota_d.ap()[:, :])
            ones_sb = res_pool.tile([1, D], BF16)
            nc.vector.memset(ones_sb[:], 1.0)

            for lay in range(2):
                src_tab = x_rep if lay == 0 else h1rep
                loc_tab = x_loc if lay == 0 else h1b

                ls = nc.enter_named_scope(f"lay_{lay}", False)
                w_sb = wpool.tile([128, (R + 1) * D], BF16, tag="w", bufs=2)
                nc.sync.dma_start(
                    out=w_sb[:].rearrange("d (r e) -> d r e", r=R + 1),
                    in_=w_all.ap()[lay].rearrange("r d e -> d r e"),
                )
                b_sb = wpool.tile([1, D], BF16, tag="b", bufs=2)
                nc.sync.dma_start(out=b_sb[:], in_=b_all.ap()[lay])

                NACC = PAIRS * B
                for bat in range(NBAT):
                    row0 = bat * B * 128
                    if B > 1 or bat % 2 == 0:
                        nrows = min(max(B, 2) * 128, NLP - row0)
                        loct = loct_pool.tile([128, max(B, 2) * 128], BF16,
                                              tag="lt")
                        nc.sync.dma_start_transpose(
                            out=loct[:, :nrows],
                            in_=loc_tab.ap()[row0 : row0 + nrows, :],
                        )
                    # accumulators, each padded to a full PSUM bank so every
                    # concurrently-open accumulation group owns its own 2KB
                    # zero region (PE start=True zeroes the whole region)
                    psA = [
                        psA_pool.tile([128, MW], F32, tag=f"pa{a}",
                                      name=f"psA{a}", bufs=2,
                                      padded_shape=[128, 512])
                        for a in range(NACC)
                    ]
                    for sb in range(NSB):
                        o, n = runs[bat][sb]
                        nch = n // 128
                        msgt = msg_pool.tile([128, nch, D], BF16, tag="msg")
                        for co in range(0, n, MAXC):
                            cn = min(MAXC, n - co)
                            nc.gpsimd.dma_gather(
                                out_ap=msgt[:, co // 128 : (co + cn) // 128, :],
                                in_ap=src_tab.ap()[sb * SBR : (sb + 1) * SBR, :],
                                idxs_ap=gidx_sb[
                                    :, (o + co) // 16 : (o + co + cn) // 16
                                ],
                                num_idxs=cn,
                                num_idxs_reg=cn,
                                elem_size=D,
                                single_packet=SINGLE_PACKET,
                                queue_num=sb % K_QUEUES,
                            )
                        use_md = (
                            K_MDRAM == 1
                            or (K_MDRAM == 2 and sb % 2 == 0)
                            or (K_MDRAM == 3 and sb != 1)
                        )
                        if use_md:
                            mt = m_pool.tile([128, nch * MW], FP8, tag="mf8")
                            nc.scalar.dma_start(
                                out=mt[:],
                                in_=m_d.ap()[
                                    :, (o // 128) * MW : ((o + n) // 128) * MW
                                ],
                            )
                        else:
                            # is_equal first: it does not depend on the
                            # gather, so it must not sit behind the
                            # gather-gated scale in the DVE queue
                            mt = m_pool.tile([128, nch * MW], BF16, tag="m16")
                            nc.vector.tensor_tensor(
                                out=mt[:].rearrange(
                                    "p (a b) -> p a b", b=MW
                                ),
                                in0=dstv_sb[:, o // 128 : (o + n) // 128, None]
                                .to_broadcast([128, nch, MW]),
                                in1=ciota_sb[:, None, :]
                                .to_broadcast([128, nch, MW]),
                                op=mybir.AluOpType.is_equal,
                            )
                        nc.vector.tensor_tensor(
                            out=msgt[:],
                            in0=msgt[:],
                            in1=scal_sb[:, o // 128 : (o + n) // 128, None]
                            .to_broadcast([128, nch, D]),
                            op=mybir.AluOpType.mult,
                        )
                        ci = 0
                        for p in range(PAIRS):
                            for t2 in range(B):
                                gi = ((bat * NSB + sb) * PAIRS + p) * B + t2
                                gch = gsz[gi] // 128
                                for k in range(gch):
                                    nc.tensor.matmul(
                                        out=psA[p * B + t2][:],
                                        lhsT=msgt[:, ci, :],
                                        rhs=mt[:, ci * MW : (ci + 1) * MW],
                                        start=(sb == 0 and k == 0),
                                        stop=(sb == NSB - 1 and k == gch - 1),
                                    )
                                    ci += 1
                        assert ci == nch

                    a_sb = [
                        a_pool.tile([128, MW], BF16, tag=f"a{a}",
                                    name=f"a_sb{a}", bufs=2)
                        for a in range(NACC)
                    ]
                    for a in range(NACC):
                        nc.scalar.activation(
                            out=a_sb[a][:], in_=psA[a][:],
                            func=mybir.ActivationFunctionType.Copy,
                        )

                    for t2 in range(B):
                        # transform reuses a late accumulator's bank (its
                        # group is closed and its data copied to SBUF by now)
                        ps = psA[(PAIRS - 1) * B + t2][:, 0:D]
                        for r in range(R):
                            a0 = (r % K_GRP) * 128
                            nc.tensor.matmul(
                                out=ps,
                                lhsT=a_sb[(r // K_GRP) * B + t2][:, a0 : a0 + 128],
                                rhs=w_sb[:, r * D : (r + 1) * D],
                                start=(r == 0),
                                stop=False,
                            )
                        lc = (bat % 2) * 128 if B == 1 else t2 * 128
                        nc.tensor.matmul(
                            out=ps,
                            lhsT=loct[:, lc : lc + 128],
                            rhs=w_sb[:, R * D : (R + 1) * D],
                            start=False,
                            stop=False,
                        )
                        nc.tensor.matmul(
                            out=ps,
                            lhsT=ones_sb[:1, :],
                            rhs=b_sb[:1, :],
                            start=False,
                            stop=True,
                        )
                        row = row0 + t2 * 128
                        if lay == 0:
                            hs = hpool.tile([128, D], BF16, tag="h0")
                            nc.scalar.activation(
                                out=hs[:], in_=ps,
                                func=mybir.ActivationFunctionType.Relu,
                            )
                            nc.sync.dma_start(
                                out=h1b.ap()[row : row + 128, :], in_=hs[:]
                            )
                        else:
                            nrow = min(128, NL - row)
                            if nrow <= 0:
                                continue
                            hs = hpool.tile([128, D], F32, tag="h1")
                            nc.scalar.activation(
                                out=hs[:], in_=ps,
                                func=mybir.ActivationFunctionType.Relu,
                            )
                            nc.sync.dma_start(
                                out=out_d.ap()[row : row + nrow, :],
                                in_=hs[:nrow, :],
                            )

                nc.leave_named_scope(f"lay_{lay}", ls[0], False)
                if lay == 0:
                    nc.gpsimd.collective_compute(
                        "AllGather",
                        mybir.AluOpType.bypass,
                        replica_groups=[list(range(C))],
                        ins=[h1b.ap()],
                        outs=[h1rep.ap()],
                    )

    nc.compile()
    return nc


# ----------------------------------------------------------------------------
# In-map assembly
# ----------------------------------------------------------------------------
def make_in_maps(prep, W1, root1, b1, W2, root2, b2, cfg):
    C, D, R = cfg["C"], cfg["D"], cfg["R"]
    NL, NT, NLP, NTOT, B, NBAT, PAIRS, NSB, SBR, MW = derive(cfg)
    w_all = np.zeros((2, R + 1, D, D), bf16)
    w_all[0, :R] = np.asarray(W1, np.float32).astype(bf16)
    w_all[0, R] = np.asarray(root1, np.float32).astype(bf16)
    w_all[1, :R] = np.asarray(W2, np.float32).astype(bf16)
    w_all[1, R] = np.asarray(root2, np.float32).astype(bf16)
    b_stack = np.stack([np.asarray(b1, np.float32), np.asarray(b2, np.float32)])
    b_all = b_stack.reshape(2, 1, D).astype(bf16)
    ciota = np.tile(np.arange(MW, dtype=np.float32).astype(bf16), (128, 1))

    in_maps = []
    for c in range(C):
        x_loc = np.ascontiguousarray(prep["x_rep"][c * NLP : (c + 1) * NLP])
        im = {
            "x_rep": prep["x_rep"],
            "x_loc": x_loc,
            "w_all": w_all,
            "b_all": b_all,
            "gidx": prep["gidx"][c],
            "dstv": prep["dstv"][c],
            "scal": prep["scal"][c],
            "ciota": ciota,
        }
        if K_MDRAM:
            im["m_w"] = prep["m_w"][c]
        in_maps.append(im)
    return in_maps


def enable_ntff_hook():
    """Register the axon NTFF profiling hook if the image's antenv lacks it."""
    import sys, types
    try:
        import antenv.axon_hooks  # noqa: F401
        return True
    except ImportError:
        pass
    try:
        from trn_agent_boot.trn_boot import _ntff_profile_via_ctypes
        hook = _ntff_profile_via_ctypes("/opt/axon/libaxon_pjrt.so")
        mod = types.ModuleType("antenv.axon_hooks")
        mod._hook = hook
        mod.set_axon_ntff_profile_hook = lambda h: setattr(mod, "_hook", h)
        mod.get_axon_ntff_profile_hook = lambda: mod._hook
        sys.modules["antenv.axon_hooks"] = mod
        import antenv
        antenv.axon_hooks = mod
        return hook is not None
    except Exception:
        return False


_program_cache = {}


def run(x, edge_index, edge_type, W1, root1, b1, W2, root2, b2,
        cfg=FULL, trace=False):
    prep = host_prep(x, edge_index, edge_type, cfg)
    key = (tuple(sorted(cfg.items())), prep["gsz"], prep["runs"], prep["PAD"],
           K_QUEUES, SINGLE_PACKET, MAXC, K_MDRAM, K_GRP, K_B)
    if key not in _program_cache:
        _program_cache[key] = build_program(
            cfg, prep["gsz"], prep["runs"], prep["PAD"]
        )
    nc = _program_cache[key]
    in_maps = make_in_maps(prep, W1, root1, b1, W2, root2, b2, cfg)
    if trace:
        trace = enable_ntff_hook()
    res = run_bass_kernel_spmd(
        nc, in_maps, core_ids=list(range(cfg["C"])), trace=trace
    )
    blocks = [res.results[c]["out"] for c in range(cfg["C"])]
    full = np.concatenate(blocks, axis=0).astype(np.float32)
    return full, res


def kernel(**inputs):
    out, _ = run(
        inputs["x"], inputs["edge_index"], inputs["edge_type"],
        inputs["W1"], inputs["root1"], inputs["b1"],
        inputs["W2"], inputs["root2"], inputs["b2"],
    )
    return out
